# revision 17
# baseline (speedup 1.0000x reference)
"""Fused NonLocalBlock2D kernel for Trainium2 (8 NeuronCores, batch-parallel).

Per-core computation (one batch sample, C=64, C2=32, N=64*64=4096):
  xf  = x[b]                          [C, N]
  f   = xf^T xf                       [N, N]   (symmetric, never in HBM)
  p   = softmax(f, axis=-1)
  gx  = g_w xf + g_b                  [N, C2]
  y   = p gx                          [N, C2]
  z   = W_w y^T + W_b + xf            [C, N]

Tricks:
  - g_b folds into b_eff = W_w g_b + W_b because softmax rows sum to 1.
  - Numerical shift: subtract D[n] = sum_c xf[c,n]^2 (the diagonal of f)
    per-column before exp; any per-n constant cancels in y = num/den.
    Realized inside the score matmul with K=65: row 64 of lhsT is ones,
    row 64 of rhs is -D.
  - Row sums d[n] obtained from the same accumulation matmul by adding a
    33rd ones-column to the gx stationary operand (via an extended
    g_wT65 host operand whose row 64 produces an exact 1.0 column).
  - 1/d broadcast across partitions via a K=1 PE matmul with a ones row.
  - All PE operands are float32r (tf32-like 1+8+11): 1 cycle/row when
    the moving free dim >= 512.  HW requires every f32r operand to be
    *written* as f32r by its producer (DVE/ACT convert on writeback);
    fp32 data used by DVE (residual add, reciprocal) is kept in
    separate fp32 tiles.
"""

import numpy as np

_REPO = "/opt/trn_rl_repo"

C = 64
C2 = 32
N = 4096
MC = 128          # m-chunk width (partition dim of E tiles)
NMC = N // MC     # 32 m-chunks
QW = 1024         # n-quarter width (PSUM: 2 banks)
NQ = N // QW      # 4 quarters
HB = 512          # half-quarter / psum-bank width

_CACHE = {}


def _ensure_path():
    import sys
    if _REPO not in sys.path:
        sys.path.insert(0, _REPO)


def _build_nc():
    _ensure_path()
    import concourse.tile as tile
    from concourse import bacc, mybir
    from contextlib import ExitStack

    fp32 = mybir.dt.float32
    f32r = mybir.dt.float32r
    fp16 = mybir.dt.float16
    bf16 = mybir.dt.bfloat16
    AF = mybir.ActivationFunctionType
    ALU = mybir.AluOpType

    nc = bacc.Bacc(
        "TRN2",
        target_bir_lowering=False,
        debug=False,
        enable_asserts=True,
        num_devices=8,
    )

    xf_d = nc.dram_tensor("xf", [C, N], fp32, kind="ExternalInput").ap()
    gwT_d = nc.dram_tensor("g_wT65", [C + 1, 33], fp32, kind="ExternalInput").ap()
    WwT_d = nc.dram_tensor("W_wT", [C2, C], fp32, kind="ExternalInput").ap()
    beff_d = nc.dram_tensor("b_eff", [C, 1], fp32, kind="ExternalInput").ap()
    out_d = nc.dram_tensor("out", [C, N], fp32, kind="ExternalOutput").ap()

    with tile.TileContext(nc) as tc, ExitStack() as ctx:
        persist = ctx.enter_context(tc.tile_pool(name="persist", bufs=1))
        xfo = persist.tile([C + 1, N], fp32)     # rows 0..63 xf, row 64 = 1.0
        xfoR = persist.tile([C + 1, N], f32r)    # f32r copy (S-mm stationary)
        xfdR = persist.tile([C + 1, N], f32r)    # rows 0..63 xf, row 64 = -D
        xsqR = persist.tile([C, N], f32r)
        gxR = persist.tile([MC, 33 * NMC], fp16)
        gwT_s = persist.tile([C + 1, 33], fp32)
        WwT_f = persist.tile([C2, C], fp32)
        WwT_R = persist.tile([C2, C], f32r)
        beff_s = persist.tile([C, 1], fp32)
        ones1f = persist.tile([1, C2], fp32)
        ones1R = persist.tile([1, C2], f32r)
        negonf = persist.tile([C, 1], fp32)
        negonR = persist.tile([C, 1], f32r)

        nc.sync.dma_start(xfo[0:C, :], xf_d)
        nc.sync.dma_start(gwT_s[:], gwT_d)
        nc.sync.dma_start(WwT_f[:], WwT_d)
        nc.sync.dma_start(beff_s[:], beff_d)
        nc.any.memset(xfo[C : C + 1, :], 1.0)
        nc.any.memset(ones1f[:], 1.0)
        nc.any.memset(negonf[:], -1.0)

        nc.vector.tensor_copy(ones1R[:], ones1f[:])
        nc.vector.tensor_copy(negonR[:], negonf[:])
        nc.vector.tensor_copy(WwT_R[:], WwT_f[:])
        nc.scalar.activation(xfoR[:], xfo[:], AF.Copy)
        nc.vector.tensor_copy(xfdR[0:C, :], xfo[0:C, :])
        nc.gpsimd.tensor_mul(xsqR[:], xfo[0:C, :], xfo[0:C, :])

        s_pool = ctx.enter_context(tc.tile_pool(name="spsum", bufs=2, space="PSUM"))
        y0_pool = ctx.enter_context(tc.tile_pool(name="y0psum", bufs=1, space="PSUM"))
        rbc_pool = ctx.enter_context(tc.tile_pool(name="rbcpsum", bufs=1, space="PSUM"))
        z_pool = ctx.enter_context(tc.tile_pool(name="zpsum", bufs=1, space="PSUM"))

        # -D[n] into xfdR row 64
        for j in range(N // HB):
            dp = s_pool.tile([1, HB], fp32, tag="S")
            nc.tensor.matmul(
                dp[:],
                lhsT=negonR[:],
                rhs=xsqR[:, j * HB : (j + 1) * HB],
                start=True,
                stop=True,
            )
            nc.vector.tensor_copy(xfdR[C : C + 1, j * HB : (j + 1) * HB], dp[:])

        # gx (33rd column == 1.0 via g_wT65 row 64), plain fp32 matmul
        for q in range(NMC):
            gp = s_pool.tile([MC, 33], fp32, tag="S")
            nc.tensor.matmul(
                gp[:],
                lhsT=xfo[:, q * MC : (q + 1) * MC],
                rhs=gwT_s[:],
                start=True,
                stop=True,
            )
            nc.vector.tensor_copy(gxR[:, q * 33 : (q + 1) * 33], gp[:])

        e_pool = ctx.enter_context(tc.tile_pool(name="e", bufs=2))
        ysb_pool = ctx.enter_context(tc.tile_pool(name="ysb", bufs=2))
        r_pool = ctx.enter_context(tc.tile_pool(name="r", bufs=2))
        y1_pool = ctx.enter_context(tc.tile_pool(name="y1", bufs=2))
        o_pool = ctx.enter_context(tc.tile_pool(name="osb", bufs=2))

        for nq in range(NQ):
            n0 = nq * QW
            y0 = y0_pool.tile([33, QW], fp32)
            for q in range(NMC):
                s_t = s_pool.tile([MC, QW], fp32, tag="S")
                for h in range(2):
                    nc.tensor.matmul(
                        s_t[:, h * HB : (h + 1) * HB],
                        lhsT=xfoR[:, q * MC : (q + 1) * MC],
                        rhs=xfdR[:, n0 + h * HB : n0 + (h + 1) * HB],
                        start=True,
                        stop=True,
                    )
                e_t = e_pool.tile([MC, QW], bf16)
                nc.scalar.activation(e_t[:], s_t[:], AF.Exp)
                for h in range(2):
                    nc.tensor.matmul(
                        y0[:, h * HB : (h + 1) * HB],
                        lhsT=gxR[:, q * 33 : (q + 1) * 33],
                        rhs=e_t[:, h * HB : (h + 1) * HB],
                        start=(q == 0),
                        stop=(q == NMC - 1),
                    )

            y_sb = ysb_pool.tile([33, QW], fp32)
            nc.vector.tensor_copy(y_sb[:], y0[:])
            r_row = r_pool.tile([1, QW], f32r)
            with nc.allow_low_precision(reason="1/d feeds f32r broadcast matmul"):
                nc.vector.reciprocal(r_row[:], y_sb[C2 : C2 + 1, :])
            for h in range(2):
                rbc = rbc_pool.tile([C2, HB], fp32)
                nc.tensor.matmul(
                    rbc[:],
                    lhsT=ones1R[:],
                    rhs=r_row[:, h * HB : (h + 1) * HB],
                    start=True,
                    stop=True,
                )
                y1 = y1_pool.tile([C2, HB], f32r)
                nc.vector.tensor_mul(y1[:], y_sb[0:C2, h * HB : (h + 1) * HB], rbc[:])
                z_t = z_pool.tile([C, HB], fp32)
                nc.tensor.matmul(
                    z_t[:],
                    lhsT=WwT_R[:],
                    rhs=y1[:],
                    start=True,
                    stop=True,
                )
                o_t = o_pool.tile([C, HB], fp32)
                nc.vector.scalar_tensor_tensor(
                    o_t[:],
                    z_t[:],
                    beff_s[:],
                    xfo[0:C, n0 + h * HB : n0 + (h + 1) * HB],
                    op0=ALU.add,
                    op1=ALU.add,
                )
                nc.sync.dma_start(out_d[:, n0 + h * HB : n0 + (h + 1) * HB], o_t[:])

    nc.compile()
    return nc


def _get_nc():
    if "nc" not in _CACHE:
        _CACHE["nc"] = _build_nc()
    return _CACHE["nc"]


def _run(inputs, trace=False, **kw):
    _ensure_path()
    from concourse.bass_utils import run_bass_kernel_spmd

    nc = _get_nc()
    x = np.ascontiguousarray(np.asarray(inputs["x"], dtype=np.float32))
    g_w = np.asarray(inputs["g_w"], dtype=np.float32)
    g_b = np.asarray(inputs["g_b"], dtype=np.float32)
    W_w = np.asarray(inputs["W_w"], dtype=np.float32)
    W_b = np.asarray(inputs["W_b"], dtype=np.float32)

    gwT65 = np.zeros((C + 1, 33), dtype=np.float32)
    gwT65[0:C, 0:C2] = g_w.T
    gwT65[C, C2] = 1.0
    WwT = np.ascontiguousarray(W_w.T)                         # [C2, C]
    b_eff = (
        W_w.astype(np.float64) @ g_b.astype(np.float64) + W_b.astype(np.float64)
    ).astype(np.float32).reshape(C, 1)

    B = x.shape[0]
    in_maps = [
        {
            "xf": np.ascontiguousarray(x[i].reshape(C, N)),
            "g_wT65": gwT65,
            "W_wT": WwT,
            "b_eff": b_eff,
        }
        for i in range(B)
    ]
    res = run_bass_kernel_spmd(nc, in_maps, list(range(B)), trace=trace, **kw)
    out = np.stack([res.results[i]["out"].reshape(C, 64, 64) for i in range(B)])
    return res, out.astype(np.float32)


def kernel(**inputs):
    _, out = _run(inputs, trace=False)
    return out


# revision 18
# speedup vs baseline: 1.3997x; 1.3997x over previous
"""Fused NonLocalBlock2D kernel for Trainium2 (8 NeuronCores, batch-parallel).

Per-core computation (one batch sample, C=64, C2=32, N=64*64=4096):
  xf  = x[b]                          [C, N]
  f   = xf^T xf                       [N, N]   (symmetric, never in HBM)
  p   = softmax(f, axis=-1)
  gx  = g_w xf + g_b                  [N, C2]
  y   = p gx                          [N, C2]
  z   = W_w y^T + W_b + xf            [C, N]

Tricks:
  - g_b folds into b_eff = W_w g_b + W_b because softmax rows sum to 1.
  - Numerical shift: subtract D[n] = sum_c xf[c,n]^2 (the diagonal of f)
    per-column before exp; any per-n constant cancels in y = num/den.
    Realized inside the score matmul with K=65: row 64 of lhsT is ones,
    row 64 of rhs is -D.
  - Row sums d[n] obtained from the same accumulation matmul by adding a
    33rd ones-column to the gx stationary operand (via an extended
    g_wT65 host operand whose row 64 produces an exact 1.0 column).
  - 1/d broadcast across partitions via a K=1 PE matmul with a ones row.
  - All PE operands are float32r (tf32-like 1+8+11): 1 cycle/row when
    the moving free dim >= 512.  HW requires every f32r operand to be
    *written* as f32r by its producer (DVE/ACT convert on writeback);
    fp32 data used by DVE (residual add, reciprocal) is kept in
    separate fp32 tiles.
"""

import numpy as np

_REPO = "/opt/trn_rl_repo"

C = 64
C2 = 32
N = 4096
MC = 128          # m-chunk width (partition dim of E tiles)
NMC = N // MC     # 32 m-chunks
QW = 1024         # n-quarter width (PSUM: 2 banks)
NQ = N // QW      # 4 quarters
HB = 512          # half-quarter / psum-bank width

_CACHE = {}


def _ensure_path():
    import sys
    if _REPO not in sys.path:
        sys.path.insert(0, _REPO)


def _build_nc():
    _ensure_path()
    import concourse.tile as tile
    from concourse import bacc, mybir
    from contextlib import ExitStack

    fp32 = mybir.dt.float32
    f32r = mybir.dt.float32r
    AF = mybir.ActivationFunctionType
    ALU = mybir.AluOpType

    nc = bacc.Bacc(
        "TRN2",
        target_bir_lowering=False,
        debug=False,
        enable_asserts=True,
        num_devices=8,
    )

    xf_d = nc.dram_tensor("xf", [C, N], fp32, kind="ExternalInput").ap()
    gwT_d = nc.dram_tensor("g_wT65", [C + 1, 33], fp32, kind="ExternalInput").ap()
    WwT_d = nc.dram_tensor("W_wT", [C2, C], fp32, kind="ExternalInput").ap()
    beff_d = nc.dram_tensor("b_eff", [C, 1], fp32, kind="ExternalInput").ap()
    out_d = nc.dram_tensor("out", [C, N], fp32, kind="ExternalOutput").ap()

    with tile.TileContext(nc) as tc, ExitStack() as ctx:
        persist = ctx.enter_context(tc.tile_pool(name="persist", bufs=1))
        xfo = persist.tile([C + 1, N], fp32)     # rows 0..63 xf, row 64 = 1.0
        xfoR = persist.tile([C + 1, N], f32r)    # f32r copy (S-mm stationary)
        xfdR = persist.tile([C + 1, N], f32r)    # rows 0..63 xf, row 64 = -D
        xsqR = persist.tile([C, N], f32r)
        gxR = persist.tile([MC, 33 * NMC], f32r)
        gwT_s = persist.tile([C + 1, 33], fp32)
        WwT_f = persist.tile([C2, C], fp32)
        WwT_R = persist.tile([C2, C], f32r)
        beff_s = persist.tile([C, 1], fp32)
        ones1f = persist.tile([1, C2], fp32)
        ones1R = persist.tile([1, C2], f32r)
        negonf = persist.tile([C, 1], fp32)
        negonR = persist.tile([C, 1], f32r)

        nc.sync.dma_start(xfo[0:C, :], xf_d)
        nc.sync.dma_start(gwT_s[:], gwT_d)
        nc.sync.dma_start(WwT_f[:], WwT_d)
        nc.sync.dma_start(beff_s[:], beff_d)
        nc.any.memset(xfo[C : C + 1, :], 1.0)
        nc.any.memset(ones1f[:], 1.0)
        nc.any.memset(negonf[:], -1.0)

        nc.vector.tensor_copy(ones1R[:], ones1f[:])
        nc.vector.tensor_copy(negonR[:], negonf[:])
        nc.vector.tensor_copy(WwT_R[:], WwT_f[:])
        nc.scalar.activation(xfoR[:], xfo[:], AF.Copy)
        nc.vector.tensor_copy(xfdR[0:C, :], xfo[0:C, :])
        nc.gpsimd.tensor_mul(xsqR[:], xfo[0:C, :], xfo[0:C, :])

        s_pool = ctx.enter_context(tc.tile_pool(name="spsum", bufs=2, space="PSUM"))
        y0_pool = ctx.enter_context(tc.tile_pool(name="y0psum", bufs=1, space="PSUM"))
        rbc_pool = ctx.enter_context(tc.tile_pool(name="rbcpsum", bufs=1, space="PSUM"))
        z_pool = ctx.enter_context(tc.tile_pool(name="zpsum", bufs=1, space="PSUM"))

        # -D[n] into xfdR row 64
        for j in range(N // HB):
            dp = s_pool.tile([1, HB], fp32, tag="S")
            nc.tensor.matmul(
                dp[:],
                lhsT=negonR[:],
                rhs=xsqR[:, j * HB : (j + 1) * HB],
                start=True,
                stop=True,
            )
            nc.vector.tensor_copy(xfdR[C : C + 1, j * HB : (j + 1) * HB], dp[:])

        # gx (33rd column == 1.0 via g_wT65 row 64), plain fp32 matmul
        for q in range(NMC):
            gp = s_pool.tile([MC, 33], fp32, tag="S")
            nc.tensor.matmul(
                gp[:],
                lhsT=xfo[:, q * MC : (q + 1) * MC],
                rhs=gwT_s[:],
                start=True,
                stop=True,
            )
            nc.vector.tensor_copy(gxR[:, q * 33 : (q + 1) * 33], gp[:])

        e_pool = ctx.enter_context(tc.tile_pool(name="e", bufs=2))
        ysb_pool = ctx.enter_context(tc.tile_pool(name="ysb", bufs=2))
        r_pool = ctx.enter_context(tc.tile_pool(name="r", bufs=2))
        y1_pool = ctx.enter_context(tc.tile_pool(name="y1", bufs=2))
        o_pool = ctx.enter_context(tc.tile_pool(name="osb", bufs=2))

        for nq in range(NQ):
            n0 = nq * QW
            y0 = y0_pool.tile([33, QW], fp32)
            for q in range(NMC):
                s_t = s_pool.tile([MC, QW], fp32, tag="S")
                for h in range(2):
                    nc.tensor.matmul(
                        s_t[:, h * HB : (h + 1) * HB],
                        lhsT=xfoR[:, q * MC : (q + 1) * MC],
                        rhs=xfdR[:, n0 + h * HB : n0 + (h + 1) * HB],
                        start=True,
                        stop=True,
                    )
                e_t = e_pool.tile([MC, QW], f32r)
                nc.scalar.activation(e_t[:], s_t[:], AF.Exp)
                for h in range(2):
                    nc.tensor.matmul(
                        y0[:, h * HB : (h + 1) * HB],
                        lhsT=gxR[:, q * 33 : (q + 1) * 33],
                        rhs=e_t[:, h * HB : (h + 1) * HB],
                        start=(q == 0),
                        stop=(q == NMC - 1),
                    )

            y_sb = ysb_pool.tile([33, QW], fp32)
            nc.vector.tensor_copy(y_sb[:], y0[:])
            r_row = r_pool.tile([1, QW], f32r)
            with nc.allow_low_precision(reason="1/d feeds f32r broadcast matmul"):
                nc.vector.reciprocal(r_row[:], y_sb[C2 : C2 + 1, :])
            for h in range(2):
                rbc = rbc_pool.tile([C2, HB], fp32)
                nc.tensor.matmul(
                    rbc[:],
                    lhsT=ones1R[:],
                    rhs=r_row[:, h * HB : (h + 1) * HB],
                    start=True,
                    stop=True,
                )
                y1 = y1_pool.tile([C2, HB], f32r)
                nc.vector.tensor_mul(y1[:], y_sb[0:C2, h * HB : (h + 1) * HB], rbc[:])
                z_t = z_pool.tile([C, HB], fp32)
                nc.tensor.matmul(
                    z_t[:],
                    lhsT=WwT_R[:],
                    rhs=y1[:],
                    start=True,
                    stop=True,
                )
                o_t = o_pool.tile([C, HB], fp32)
                nc.vector.scalar_tensor_tensor(
                    o_t[:],
                    z_t[:],
                    beff_s[:],
                    xfo[0:C, n0 + h * HB : n0 + (h + 1) * HB],
                    op0=ALU.add,
                    op1=ALU.add,
                )
                nc.sync.dma_start(out_d[:, n0 + h * HB : n0 + (h + 1) * HB], o_t[:])

    nc.compile()
    return nc


def _get_nc():
    if "nc" not in _CACHE:
        _CACHE["nc"] = _build_nc()
    return _CACHE["nc"]


def _run(inputs, trace=False, **kw):
    _ensure_path()
    from concourse.bass_utils import run_bass_kernel_spmd

    nc = _get_nc()
    x = np.ascontiguousarray(np.asarray(inputs["x"], dtype=np.float32))
    g_w = np.asarray(inputs["g_w"], dtype=np.float32)
    g_b = np.asarray(inputs["g_b"], dtype=np.float32)
    W_w = np.asarray(inputs["W_w"], dtype=np.float32)
    W_b = np.asarray(inputs["W_b"], dtype=np.float32)

    gwT65 = np.zeros((C + 1, 33), dtype=np.float32)
    gwT65[0:C, 0:C2] = g_w.T
    gwT65[C, C2] = 1.0
    WwT = np.ascontiguousarray(W_w.T)                         # [C2, C]
    b_eff = (
        W_w.astype(np.float64) @ g_b.astype(np.float64) + W_b.astype(np.float64)
    ).astype(np.float32).reshape(C, 1)

    B = x.shape[0]
    in_maps = [
        {
            "xf": np.ascontiguousarray(x[i].reshape(C, N)),
            "g_wT65": gwT65,
            "W_wT": WwT,
            "b_eff": b_eff,
        }
        for i in range(B)
    ]
    res = run_bass_kernel_spmd(nc, in_maps, list(range(B)), trace=trace, **kw)
    out = np.stack([res.results[i]["out"].reshape(C, 64, 64) for i in range(B)])
    return res, out.astype(np.float32)


def kernel(**inputs):
    _, out = _run(inputs, trace=False)
    return out


# revision 20
# speedup vs baseline: 1.4305x; 1.0220x over previous
"""Fused NonLocalBlock2D kernel for Trainium2 (8 NeuronCores, batch-parallel).

Per-core computation (one batch sample, C=64, C2=32, N=64*64=4096):
  xf  = x[b]                          [C, N]
  f   = xf^T xf                       [N, N]   (symmetric, never in HBM)
  p   = softmax(f, axis=-1)
  gx  = g_w xf + g_b                  [N, C2]
  y   = p gx                          [N, C2]
  z   = W_w y^T + W_b + xf            [C, N]

Tricks:
  - g_b folds into b_eff = W_w g_b + W_b because softmax rows sum to 1.
  - Numerical shift: subtract D[n] = sum_c xf[c,n]^2 (the diagonal of f)
    per-column before exp; any per-n constant cancels in y = num/den.
    Realized inside the score matmul with K=65: row 64 of lhsT is ones,
    row 64 of rhs is -D.
  - Row sums d[n] obtained from the same accumulation matmul by adding a
    33rd ones-column to the gx stationary operand (via an extended
    g_wT65 host operand whose row 64 produces an exact 1.0 column).
  - 1/d broadcast across partitions via a K=1 PE matmul with a ones row.
  - All PE operands are float32r (tf32-like 1+8+11): 1 cycle/row when
    the moving free dim >= 512.  HW requires every f32r operand to be
    *written* as f32r by its producer (DVE/ACT convert on writeback);
    fp32 data used by DVE (residual add, reciprocal) is kept in
    separate fp32 tiles.
"""

import numpy as np

_REPO = "/opt/trn_rl_repo"

C = 64
C2 = 32
N = 4096
MC = 128          # m-chunk width (partition dim of E tiles)
NMC = N // MC     # 32 m-chunks
QW = 1024         # n-quarter width (PSUM: 2 banks)
NQ = N // QW      # 4 quarters
HB = 512          # half-quarter / psum-bank width

_CACHE = {}


def _ensure_path():
    import sys
    if _REPO not in sys.path:
        sys.path.insert(0, _REPO)


def _build_nc():
    _ensure_path()
    import concourse.tile as tile
    from concourse import bacc, mybir
    from contextlib import ExitStack

    fp32 = mybir.dt.float32
    f32r = mybir.dt.float32r
    AF = mybir.ActivationFunctionType
    ALU = mybir.AluOpType

    nc = bacc.Bacc(
        "TRN2",
        target_bir_lowering=False,
        debug=False,
        enable_asserts=True,
        num_devices=8,
    )

    xf_d = nc.dram_tensor("xf", [C, N], fp32, kind="ExternalInput").ap()
    gwT_d = nc.dram_tensor("g_wT65", [C + 1, 33], fp32, kind="ExternalInput").ap()
    WwT_d = nc.dram_tensor("W_wT", [C2, C], fp32, kind="ExternalInput").ap()
    beff_d = nc.dram_tensor("b_eff", [C, 1], fp32, kind="ExternalInput").ap()
    out_d = nc.dram_tensor("out", [C, N], fp32, kind="ExternalOutput").ap()

    with tile.TileContext(nc) as tc, ExitStack() as ctx:
        persist = ctx.enter_context(tc.tile_pool(name="persist", bufs=1))
        xfo = persist.tile([C + 1, N], fp32)     # rows 0..63 xf, row 64 = 1.0
        xfoR = persist.tile([C + 1, N], f32r)    # f32r copy (S-mm stationary)
        xfdR = persist.tile([C + 1, N], f32r)    # rows 0..63 xf, row 64 = -D
        xsqR = persist.tile([C, N], f32r)
        gxR = persist.tile([MC, 33 * NMC], f32r)
        gwT_s = persist.tile([C + 1, 33], fp32)
        WwT_f = persist.tile([C2, C], fp32)
        WwT_R = persist.tile([C2, C], f32r)
        beff_s = persist.tile([C, 1], fp32)
        ones1f = persist.tile([1, C2], fp32)
        ones1R = persist.tile([1, C2], f32r)
        negonf = persist.tile([C, 1], fp32)
        negonR = persist.tile([C, 1], f32r)

        nc.sync.dma_start(xfo[0:C, :], xf_d)
        nc.sync.dma_start(gwT_s[:], gwT_d)
        nc.sync.dma_start(WwT_f[:], WwT_d)
        nc.sync.dma_start(beff_s[:], beff_d)
        nc.any.memset(xfo[C : C + 1, :], 1.0)
        nc.any.memset(ones1f[:], 1.0)
        nc.any.memset(negonf[:], -1.0)

        nc.vector.tensor_copy(ones1R[:], ones1f[:])
        nc.vector.tensor_copy(negonR[:], negonf[:])
        nc.vector.tensor_copy(WwT_R[:], WwT_f[:])
        nc.scalar.activation(xfoR[:], xfo[:], AF.Copy)
        nc.vector.tensor_copy(xfdR[0:C, :], xfo[0:C, :])
        nc.gpsimd.tensor_mul(xsqR[:], xfo[0:C, :], xfo[0:C, :])

        s_pool = ctx.enter_context(tc.tile_pool(name="spsum", bufs=2, space="PSUM"))
        y0_pool = ctx.enter_context(tc.tile_pool(name="y0psum", bufs=1, space="PSUM"))
        rbc_pool = ctx.enter_context(tc.tile_pool(name="rbcpsum", bufs=1, space="PSUM"))
        z_pool = ctx.enter_context(tc.tile_pool(name="zpsum", bufs=1, space="PSUM"))

        # -D[n] into xfdR row 64
        for j in range(N // HB):
            dp = s_pool.tile([1, HB], fp32, tag="S")
            nc.tensor.matmul(
                dp[:],
                lhsT=negonR[:],
                rhs=xsqR[:, j * HB : (j + 1) * HB],
                start=True,
                stop=True,
            )
            nc.vector.tensor_copy(xfdR[C : C + 1, j * HB : (j + 1) * HB], dp[:])

        # gx (33rd column == 1.0 via g_wT65 row 64), plain fp32 matmul
        for q in range(NMC):
            gp = s_pool.tile([MC, 33], fp32, tag="S")
            nc.tensor.matmul(
                gp[:],
                lhsT=xfo[:, q * MC : (q + 1) * MC],
                rhs=gwT_s[:],
                start=True,
                stop=True,
            )
            nc.vector.tensor_copy(gxR[:, q * 33 : (q + 1) * 33], gp[:])

        e_pool = ctx.enter_context(tc.tile_pool(name="e", bufs=2))
        ysb_pool = ctx.enter_context(tc.tile_pool(name="ysb", bufs=2))
        r_pool = ctx.enter_context(tc.tile_pool(name="r", bufs=2))
        y1_pool = ctx.enter_context(tc.tile_pool(name="y1", bufs=2))
        o_pool = ctx.enter_context(tc.tile_pool(name="osb", bufs=2))

        for nq in range(NQ):
            n0 = nq * QW
            y0 = y0_pool.tile([33, QW], fp32)
            for q in range(NMC):
                s_t = s_pool.tile([MC, QW], fp32, tag="S")
                for h in range(2):
                    nc.tensor.matmul(
                        s_t[:, h * HB : (h + 1) * HB],
                        lhsT=xfoR[:, q * MC : (q + 1) * MC],
                        rhs=xfdR[:, n0 + h * HB : n0 + (h + 1) * HB],
                        start=True,
                        stop=True,
                    )
                e_t = e_pool.tile([MC, QW], f32r)
                nc.scalar.activation(e_t[:], s_t[:], AF.Exp)
                for h in range(2):
                    nc.tensor.matmul(
                        y0[:, h * HB : (h + 1) * HB],
                        lhsT=gxR[:, q * 33 : (q + 1) * 33],
                        rhs=e_t[:, h * HB : (h + 1) * HB],
                        start=(q == 0),
                        stop=(q == NMC - 1),
                    )

            y_sb = ysb_pool.tile([33, QW], fp32)
            nc.vector.tensor_copy(y_sb[:], y0[:])
            r_row = r_pool.tile([1, QW], f32r)
            with nc.allow_low_precision(reason="1/d feeds f32r broadcast matmul"):
                nc.vector.reciprocal(r_row[:], y_sb[C2 : C2 + 1, :])
            for h in range(2):
                rbc = rbc_pool.tile([C2, HB], fp32)
                nc.tensor.matmul(
                    rbc[:],
                    lhsT=ones1R[:],
                    rhs=r_row[:, h * HB : (h + 1) * HB],
                    start=True,
                    stop=True,
                )
                y1 = y1_pool.tile([C2, HB], f32r)
                nc.vector.tensor_mul(y1[:], y_sb[0:C2, h * HB : (h + 1) * HB], rbc[:])
                z_t = z_pool.tile([C, HB], fp32)
                nc.tensor.matmul(
                    z_t[:],
                    lhsT=WwT_R[:],
                    rhs=y1[:],
                    start=True,
                    stop=True,
                )
                o_t = o_pool.tile([C, HB], fp32)
                nc.vector.scalar_tensor_tensor(
                    o_t[:],
                    z_t[:],
                    beff_s[:],
                    xfo[0:C, n0 + h * HB : n0 + (h + 1) * HB],
                    op0=ALU.add,
                    op1=ALU.add,
                )
                nc.sync.dma_start(out_d[:, n0 + h * HB : n0 + (h + 1) * HB], o_t[:])

    nc.compile()
    return nc


def _get_nc():
    if "nc" not in _CACHE:
        _CACHE["nc"] = _build_nc()
    return _CACHE["nc"]


def _run(inputs, trace=False, **kw):
    _ensure_path()
    from concourse.bass_utils import run_bass_kernel_spmd

    nc = _get_nc()
    x = np.ascontiguousarray(np.asarray(inputs["x"], dtype=np.float32))
    g_w = np.asarray(inputs["g_w"], dtype=np.float32)
    g_b = np.asarray(inputs["g_b"], dtype=np.float32)
    W_w = np.asarray(inputs["W_w"], dtype=np.float32)
    W_b = np.asarray(inputs["W_b"], dtype=np.float32)

    gwT65 = np.zeros((C + 1, 33), dtype=np.float32)
    gwT65[0:C, 0:C2] = g_w.T
    gwT65[C, C2] = 1.0
    WwT = np.ascontiguousarray(W_w.T)                         # [C2, C]
    b_eff = (
        W_w.astype(np.float64) @ g_b.astype(np.float64) + W_b.astype(np.float64)
    ).astype(np.float32).reshape(C, 1)

    B = x.shape[0]
    in_maps = [
        {
            "xf": np.ascontiguousarray(x[i].reshape(C, N)),
            "g_wT65": gwT65,
            "W_wT": WwT,
            "b_eff": b_eff,
        }
        for i in range(B)
    ]
    res = run_bass_kernel_spmd(nc, in_maps, list(range(B)), trace=trace, **kw)
    out = np.stack([res.results[i]["out"].reshape(C, 64, 64) for i in range(B)])
    return res, out.astype(np.float32)


def kernel(**inputs):
    _, out = _run(inputs, trace=False)
    return out
